# revision 47
# baseline (speedup 1.0000x reference)
"""Trainium2 Bass kernel for DKernelPredefinedSparseAttention.

Problem: B=1, S=8192, H=16, D=128 attention where each 64-wide query block
attends to <=8 key blocks given by kidx/kvalid (block-sparse pattern with
element-level causal masking inside blocks), softmax over the gathered keys.

Strategy (host-specialized):
  - Heads sharded across 8 cores (2 heads/core), SPMD program.
  - Host pre-transposes Q and K per head to [D, S] layout (d on partitions)
    so no on-chip transposes are needed; V stays s-major.
  - kidx/kvalid are host-visible => the matmul schedule is specialized to the
    pattern: k-blocks are processed in PAIRS (2t, 2t+1) stacked on the 128
    partitions; for each pair the set of attending q-blocks forms contiguous
    runs => big moving-operand matmuls (fp32r, 1 cyc/row at N>=256).
  - Scores are computed transposed: S^T[kpos, q] (kpos on partitions), the
    causal-diagonal / invalid-block masks are ADDED via tiny bf16 matmuls
    from a host-built mask library (avoids DVE passes), exp runs on ScalarE
    (PSUM->SBUF, scale=1/sqrt(D) folded in), and PV accumulates O^T[d, q] in
    PSUM with V pairs as the stationary operand.
  - Softmax denominator (lsum, default): per chunk, the P tiles are first
    accumulated on DVE (f16 copy/add into an SBUF strip; DVE has large
    slack), then ONE ones-vector matmul per chunk reduces across kpos --
    cutting the PE columns spent on l ~4.4x vs a ones-matmul per segment
    (~40us/iter measured win, numerically identical output).
  - Unnormalized O^T and the denominators l are DMA'd out; the host does the
    final transpose + division during output assembly.
"""

import math
import os
import numpy as np
import ml_dtypes

BS = 64          # sparse block size (k-block width, q-block height)
CHUNK = 512      # q columns per PSUM accumulator bank (8 q-blocks)
TS = 2048        # SBUF tensor-tile width (s positions per k/q/v tile)
NEG = -1.0e30


# ----------------------------------------------------------------------------
# host-side schedule construction
# ----------------------------------------------------------------------------

class _Tile:
    __slots__ = ("t", "q0", "q1", "width", "start_chunk", "masks",
                 "qk_pieces", "segments", "slot", "gidx")

    def __init__(self, t, q0, q1):
        self.t = t
        self.q0 = q0            # first q-block (inclusive)
        self.q1 = q1            # last q-block (inclusive)
        self.width = (q1 - q0 + 1) * BS
        self.start_chunk = (q0 * BS) // CHUNK
        self.masks = []         # (rel_block, combo_idx)
        self.qk_pieces = []     # (abs_c0, abs_c1)
        self.segments = []      # (chunk, rel0, rel1)  [rel = col within tile]


_COMBOS = [("D", "Z"), ("Z", "D"), ("D", "I"), ("I", "D"),
           ("I", "Z"), ("Z", "I"), ("I", "I")]
_COMBO_IDX = {c: i for i, c in enumerate(_COMBOS)}


def _build_consts():
    """Mask library lhsT [128,128] and combo selector rhs [128, 64*ncombo]."""
    lib = np.zeros((128, 128), np.float32)
    for r in range(63):            # row r: top-diag mask column r
        lib[r, :64] = np.where(np.arange(64) <= r, 0.0, NEG)
    for r in range(63, 126):       # row r: bottom-diag mask column r-63
        c = r - 63
        lib[r, 64:] = np.where(np.arange(64) <= c, 0.0, NEG)
    lib[126, :64] = NEG            # top-inf
    lib[127, 64:] = NEG            # bottom-inf

    sel = np.zeros((128, 64 * len(_COMBOS)), np.float32)
    for ci, (top, bot) in enumerate(_COMBOS):
        for c in range(64):
            col = ci * 64 + c
            if top == "D":
                if c < 63:
                    sel[c, col] = 1.0
            elif top == "I":
                sel[126, col] = 1.0
            if bot == "D":
                if c < 63:
                    sel[63 + c, col] = 1.0
            elif bot == "I":
                sel[127, col] = 1.0
    return (lib.astype(ml_dtypes.bfloat16), sel.astype(ml_dtypes.bfloat16))


def _build_maskc():
    """Additive mask table for the DVE path: [128 kpos, combo, 64 qcol]."""
    m = np.zeros((128, len(_COMBOS), BS), np.float32)
    pb = np.arange(64)
    for ci, (top, bot) in enumerate(_COMBOS):
        for half, st in ((0, top), (64, bot)):
            if st == "D":
                m[half:half + 64, ci, :] = np.where(
                    pb[:, None] > np.arange(BS)[None, :], NEG, 0.0)
            elif st == "I":
                m[half:half + 64, ci, :] = NEG
    return m


def _need_zerofill(groups, s):
    """True only for degenerate patterns where some chunk's PSUM columns
    are not fully covered by segments (the evac would read junk)."""
    nchunk = s // CHUNK
    cov = [np.zeros(CHUNK, dtype=bool) for _ in range(nchunk)]
    anyseg = [False] * nchunk
    for gs in groups:
        for g in gs:
            for T in g:
                for (ch, r0, r1) in T.segments:
                    a = T.q0 * BS + r0 - ch * CHUNK
                    cov[ch][a:a + (r1 - r0)] = True
                    anyseg[ch] = True
    return any(anyseg[c] and not cov[c].all() for c in range(nchunk))


def _contiguous_runs(mask):
    runs = []
    i = 0
    n = len(mask)
    while i < n:
        if mask[i]:
            j = i
            while j + 1 < n and mask[j + 1]:
                j += 1
            runs.append((i, j))
            i = j + 1
        else:
            i += 1
    return runs


def _build_allow(kidx, kvalid, nb):
    allow = np.zeros((nb, nb), dtype=bool)
    kmax = kidx.shape[1]
    for i in range(nb):
        for jj in range(kmax):
            if kvalid[i, jj]:
                j = int(kidx[i, jj])
                if 0 <= j <= i:
                    allow[i, j] = True
    return allow


def _build_schedule(allow, nb, s):
    """Build the tile list + per-chunk grouping for one head (pattern is
    shared by all heads)."""
    chunkb = CHUNK // BS          # q-blocks per chunk
    nchunk = s // CHUNK
    tiles = []
    for t in range(nb // 2):
        j0, j1 = 2 * t, 2 * t + 1
        rows = allow[:, j0] | (allow[:, j1] if j1 < nb else False)
        for (a, b) in _contiguous_runs(list(rows)):
            if (b - a + 1) > chunkb:
                p = a
                while p <= b:
                    pe = min(b, (p // chunkb + 1) * chunkb - 1)
                    tiles.append(_Tile(t, p, pe))
                    p = pe + 1
            else:
                tiles.append(_Tile(t, a, b))

    for T in tiles:
        # masks per q-block
        for q in range(T.q0, T.q1 + 1):
            states = []
            for h in range(2):
                j = 2 * T.t + h
                if j >= nb or not allow[q, j]:
                    states.append("I")
                elif j == q:
                    states.append("D")
                else:
                    states.append("Z")
            if states != ["Z", "Z"]:
                T.masks.append((q - T.q0, _COMBO_IDX[tuple(states)]))
        # qk pieces: split [q0*BS, (q1+1)*BS) at the TS grid
        c0 = T.q0 * BS
        c1 = (T.q1 + 1) * BS
        p = c0
        while p < c1:
            pe = min(c1, (p // TS + 1) * TS)
            T.qk_pieces.append((p, pe))
            p = pe
        # segments: split at the CHUNK grid (for l and O^T accumulation)
        p = c0
        while p < c1:
            pe = min(c1, (p // CHUNK + 1) * CHUNK)
            T.segments.append((p // CHUNK, p - c0, pe - c0))
            p = pe

    # group tiles per start chunk (pairs, matched widths when possible)
    by_chunk = [[] for _ in range(nchunk)]
    for T in tiles:
        by_chunk[T.start_chunk].append(T)
    groups = [[] for _ in range(nchunk)]
    for c in range(nchunk):
        ts_sorted = sorted(by_chunk[c], key=lambda T: -T.width)
        for i in range(0, len(ts_sorted), 2):
            g = ts_sorted[i:i + 2]
            for slot, T in enumerate(g):
                T.slot = slot
            groups[c].append(g)

    # contribution counts per chunk (same for O^T and l)
    n_ot = [0] * nchunk
    for c in range(nchunk):
        for g in groups[c]:
            for T in g:
                for (ch, r0, r1) in T.segments:
                    n_ot[ch] += 1
    return tiles, groups, n_ot


# ----------------------------------------------------------------------------
# device program emission
# ----------------------------------------------------------------------------

def _emit_program(groups, n_ot, s, hpc, n_cores, repeat=1):
    import concourse.bacc as bacc
    import concourse.tile as tile
    import concourse.mybir as mybir
    from contextlib import ExitStack

    f32 = mybir.dt.float32
    f32r = mybir.dt.float32r
    bf16 = mybir.dt.bfloat16
    f16 = mybir.dt.float16
    Exp = mybir.ActivationFunctionType.Exp

    nchunk = s // CHUNK
    nt = math.ceil(s / TS)            # tensor tiles per head
    nlblk = math.ceil(nchunk / 4)
    d = 128
    scale = 1.0 / math.sqrt(float(d))

    variant = os.environ.get("K_VARIANT", "f16pv,lsum,lag1")
    vset = set(variant.split(","))
    nc = bacc.Bacc("TRN2", debug=False, num_devices=n_cores)
    f8 = mybir.dt.float8e4
    qkdt = f16 if "f16pv" in vset else f32r
    if "qk8" in vset:
        qkdt = f8
    QT = nc.dram_tensor("QT", [hpc, d, s], qkdt, kind="ExternalInput").ap()
    KT = nc.dram_tensor("KT", [hpc, d, s], qkdt, kind="ExternalInput").ap()
    V = None
    if "f16pv" not in vset:
        V = nc.dram_tensor("V", [hpc, s, d], f32r, kind="ExternalInput").ap()
    # consts only some variants need: keep them out of the startup DMA
    # critical path otherwise
    need_pconst = bool(vset & {"noqk", "noexp"})
    need_maskc = "dvemask" in vset
    need_zero = _need_zerofill(groups, s)

    MASKLIB = nc.dram_tensor("MASKLIB", [128, 128], bf16, kind="ExternalInput").ap()
    MASKC = None
    if need_maskc:
        MASKC = nc.dram_tensor("MASKC", [128, len(_COMBOS), BS], f32,
                               kind="ExternalInput").ap()
    COMBOS = nc.dram_tensor("COMBOS", [128, 64 * len(_COMBOS)], bf16,
                            kind="ExternalInput").ap()
    ONES = nc.dram_tensor("ONES", [128, 1], f32r, kind="ExternalInput").ap()
    ZEROSC = None
    if need_zero:
        ZEROSC = nc.dram_tensor("ZEROSC", [128, CHUNK], f32r,
                                kind="ExternalInput").ap()
    # pre-rearranged on host to [hpc, p=128, a, d] so the DMA is contiguous
    pvdt = f8 if "pv8" in vset else f16
    VF16 = nc.dram_tensor("VF16", [hpc, 128, s // 128, d], pvdt,
                          kind="ExternalInput").ap()
    ONES16 = nc.dram_tensor("ONES16", [128, 1], pvdt,
                            kind="ExternalInput").ap()
    PCONST = None
    if need_pconst:
        PCONST = nc.dram_tensor("PCONST", [128, 2, CHUNK], f16,
                                kind="ExternalInput").ap()
    otdt = f16 if "f16pv" in vset else f32
    OT = nc.dram_tensor("OT", [hpc, d, s], otdt, kind="ExternalOutput").ap()
    LOUT = nc.dram_tensor("LOUT", [hpc, s], f32, kind="ExternalOutput").ap()

    # ---- load plan on the global (head, chunk) axis ----
    # first need (global chunk) per (head, kind, tile_n)
    first_need = {}
    for h in range(hpc):
        for c in range(nchunk):
            gc = h * nchunk + c
            for g in groups[c]:
                for T in g:
                    for (p0, p1) in T.qk_pieces:
                        key = (h, "q", p0 // TS)
                        first_need.setdefault(key, gc)
                    first_need.setdefault((h, "k", (T.t * 128) // TS), gc)
                    first_need.setdefault((h, "v", (T.t * 128) // TS), gc)
    dist = 1
    for v_ in vset:
        if v_.startswith("dist"):
            dist = int(v_[4:])
    loads_at = [[] for _ in range(hpc * nchunk)]
    for (h, kind, n), gc in sorted(first_need.items(), key=lambda kv: kv[1]):
        loads_at[max(0, gc - dist)].append((h, kind, n))
    if "prefetchall" in vset:
        loads_at = [sum(loads_at, [])] + [[] for _ in range(hpc * nchunk - 1)]

    with tile.TileContext(nc) as tc, ExitStack() as ctx:
        const_pool = ctx.enter_context(tc.tile_pool(name="consts", bufs=1))
        nres = hpc * math.ceil(s / TS)      # tiles per kind if fully resident
        pall = "prefetchall" in vset
        kq_pool = ctx.enter_context(tc.tile_pool(
            name="kq", bufs=nres if pall else 5))
        q_pool = ctx.enter_context(tc.tile_pool(
            name="qp", bufs=nres if pall else (5 if "loadonce" in vset else 3)))
        v_pool = ctx.enter_context(tc.tile_pool(
            name="vp", bufs=nres if pall else 5))
        pt_pool = ctx.enter_context(tc.tile_pool(
            name="pt", bufs=8 if "lag1" in vset
            else (6 if "deep" in vset else 4)))
        stg_pool = ctx.enter_context(tc.tile_pool(
            name="stg", bufs=4 if "deep" in vset else 2))
        ps_pool = ctx.enter_context(tc.tile_pool(name="ps", bufs=1, space="PSUM"))

        masklib = const_pool.tile([128, 128], bf16)
        combos = const_pool.tile([128, 64 * len(_COMBOS)], bf16)
        ones = const_pool.tile([128, 1], f32r)
        ones16 = const_pool.tile([128, 1], pvdt)
        nc.sync.dma_start(masklib[:], MASKLIB)
        nc.sync.dma_start(combos[:], COMBOS)
        nc.sync.dma_start(ones[:], ONES)
        nc.sync.dma_start(ones16[:], ONES16)
        zerosc = pconst = maskc = None
        if need_zero:
            zerosc = const_pool.tile([128, CHUNK], f32r)
            nc.sync.dma_start(zerosc[:], ZEROSC)
        if need_pconst:
            pconst = const_pool.tile([128, 2, CHUNK], f16)
            nc.sync.dma_start(pconst[:], PCONST)
        if need_maskc:
            maskc = const_pool.tile([128, len(_COMBOS), BS], f32)
            nc.sync.dma_start(maskc[:], MASKC)

        ptdt = f16 if "f16pv" in vset else f32r
        if "pv8" in vset:
            ptdt = f8
        kt_tiles = [[None] * nt for _ in range(hpc)]
        qt_tiles = [[None] * nt for _ in range(hpc)]
        v_tiles = [[None] * nt for _ in range(hpc)]
        v16_tiles = [[None] * nt for _ in range(hpc)]

        def do_loads(gc):
            for (h, kind, n) in loads_at[gc]:
                if "loadonce" in vset:
                    done = {"k": kt_tiles, "q": qt_tiles, "v": v16_tiles}[kind]
                    if done[h][n] is not None:
                        continue
                w = min(TS, s - n * TS)
                nsplit = 4 if ("splitload" in vset and gc == 0) else 1
                if kind == "k":
                    kt = kq_pool.tile([128, w], qkdt, tag="kt", name=f"kt{h}_{n}")
                    for sp in range(nsplit):
                        a, b = sp * w // nsplit, (sp + 1) * w // nsplit
                        nc.sync.dma_start(
                            kt[:, a:b], KT[h][:, n * TS + a:n * TS + b])
                    kt_tiles[h][n] = kt
                elif kind == "q":
                    qt = q_pool.tile([128, w], qkdt, tag="qt", name=f"qt{h}_{n}")
                    for sp in range(nsplit):
                        a, b = sp * w // nsplit, (sp + 1) * w // nsplit
                        nc.sync.dma_start(
                            qt[:, a:b], QT[h][:, n * TS + a:n * TS + b])
                    qt_tiles[h][n] = qt
                else:
                    if "f16pv" not in vset:
                        vt = v_pool.tile([128, w // 128, 128], f32r, tag="vt",
                                         name=f"vt{h}_{n}")
                        src = V[h][n * TS:n * TS + w, :].rearrange(
                            "(a p) d -> p a d", p=128)
                        nc.sync.dma_start(vt[:], src)
                        v_tiles[h][n] = vt
                    a0 = (n * TS) // 128
                    vt16 = v_pool.tile([128, w // 128, 128], pvdt, tag="vt16",
                                       name=f"vt16_{h}_{n}")
                    nc.sync.dma_start(vt16[:], VF16[h][:, a0:a0 + w // 128, :])
                    v16_tiles[h][n] = vt16

        for rep in range(repeat):
          for h in range(hpc):
            ot_ps = [None] * nchunk
            l_ps = [None] * nchunk
            pending = []          # (T, pgrp) tiles with unemitted segments

            l_strip = None
            if not (vset & {"dmaonly", "nosum", "noout"}):
                l_strip = stg_pool.tile([1, s], f32, tag="lstrip", bufs=2,
                                        name=f"lstrip{h}")
            # lag1: close chunk c's PV/l block only after chunk c+1's
            # QK/exp have been emitted, so PE never stalls on the last
            # group's exp at a chunk boundary
            lag = 1 if "lag1" in vset else 0
            for c in range(nchunk + lag):
                if c < nchunk:
                    do_loads(h * nchunk + c)
                if "dmaonly" in vset:
                    continue

                for g in (groups[c] if c < nchunk else []):
                    sgrp = None
                    if "noqk" not in vset:
                        sgrp = ps_pool.tile([128, 2, CHUNK], f32, tag="sg",
                                            bufs=2, name=f"sg{h}_{c}")
                    pgrp = None
                    if "noexp" not in vset:
                        pgrp = pt_pool.tile([128, 2, CHUNK], ptdt, tag="pg",
                                            name=f"pg{h}_{c}")

                    def _mm(T):
                        """Masks needing a PE matmul (vs DVE memset on P)."""
                        if vset & {"nomask", "noqk", "dvemask"}:
                            return []
                        if "zmask" in vset:
                            return [m for m in T.masks
                                    if "D" in _COMBOS[m[1]]]
                        return T.masks

                    # QK pieces
                    for T in g:
                        npiece = len(T.qk_pieces)
                        if "noqk" in vset:
                            npiece = 0
                        for pi, (p0, p1) in enumerate(T.qk_pieces[:npiece]):
                            n = p0 // TS
                            rel = p0 - T.q0 * BS
                            kt = kt_tiles[h][T.t // (TS // 128)]
                            krel = (T.t * 128) % TS
                            qt = qt_tiles[h][n]
                            nc.tensor.matmul(
                                sgrp[:, T.slot, rel:rel + (p1 - p0)],
                                kt[:, krel:krel + 128],
                                qt[:, p0 - n * TS:p1 - n * TS],
                                start=(pi == 0),
                                stop=(pi == npiece - 1 and not _mm(T)),
                                skip_group_check=True)
                    # masks (shared masklib stationary)
                    zmask_list = []
                    for T in g:
                        nmask = len(T.masks)
                        if vset & {"nomask", "noqk"}:
                            nmask = 0
                        if "dvemask" in vset:
                            for (rb, ci) in T.masks[:nmask]:
                                dst = sgrp[:, T.slot, rb * BS:(rb + 1) * BS]
                                nc.vector.tensor_add(dst, dst,
                                                     maskc[:, ci, :])
                            continue
                        mm = _mm(T)
                        if nmask and "zmask" in vset:
                            zmask_list += [(T, rb, _COMBOS[ci])
                                           for (rb, ci) in T.masks
                                           if "D" not in _COMBOS[ci]]
                        for mi, (rb, ci) in enumerate(mm[:nmask]):
                            nc.tensor.matmul(
                                sgrp[:, T.slot, rb * BS:(rb + 1) * BS],
                                masklib[:],
                                combos[:, ci * BS:(ci + 1) * BS],
                                start=False, stop=(mi == len(mm) - 1),
                                skip_group_check=True)
                    # exp (one call per uniform-width sub-run)
                    src = pconst if "noqk" in vset else sgrp
                    if "noexp" in vset:
                        pgrp = pconst
                    elif (len(g) == 2 and g[0].width == g[1].width
                          and "splitexp" not in vset):
                        w = g[0].width
                        nc.scalar.activation(pgrp[:, :, :w], src[:, :, :w],
                                             Exp, scale=scale)
                    else:
                        for T in g:
                            nc.scalar.activation(
                                pgrp[:, T.slot, :T.width],
                                src[:, T.slot, :T.width], Exp, scale=scale)
                    # pure-invalid halves: zero P on DVE instead of a -inf
                    # mask matmul on PE (exp of the unmasked scores is junk
                    # but finite; zeroing excludes it from l and PV)
                    if "noexp" not in vset:
                        for (T, rb, (top, bot)) in zmask_list:
                            c0, c1 = rb * BS, (rb + 1) * BS
                            if top == "I" and bot == "I":
                                nc.vector.memset(
                                    pgrp[:, T.slot, c0:c1], 0.0)
                            elif top == "I":
                                nc.vector.memset(
                                    pgrp[0:64, T.slot, c0:c1], 0.0)
                            elif bot == "I":
                                nc.vector.memset(
                                    pgrp[64:128, T.slot, c0:c1], 0.0)
                    pf16 = None
                    needs16 = [T for T in g
                               if any((r1 - r0) < 256 and len(T.segments) > 1
                                      for (ch, r0, r1) in T.segments)]
                    if "f16pv" in vset:
                        needs16 = []
                    if needs16:
                        pf16 = pt_pool.tile([128, 2, CHUNK], f16, tag="pf16",
                                            bufs=4, name=f"pf16_{h}_{c}")
                        for T in needs16:
                            nc.vector.tensor_copy(
                                pf16[:, T.slot, :T.width],
                                pgrp[:, T.slot, :T.width])
                    for T in g:
                        pending.append((T, pgrp, pf16))

                # ---- close chunk c-lag: emit its l/PV segments ----
                c = c - lag
                if c < 0:
                    continue
                segs_c = []
                for (T, pgrp, pf16_) in pending:
                    for (ch, r0, r1) in T.segments:
                        if ch == c:
                            segs_c.append((T, pgrp, pf16_, r0, r1))
                pending = [(T, p, pf) for (T, p, pf) in pending
                           if any(ch > c for (ch, _, _) in T.segments)]
                if "segsort" in vset:
                    # widest first: the lsum DVE pass then does one
                    # full-width copy + one add per later segment
                    segs_c.sort(key=lambda s: -(s[4] - s[3]))

                use_lsum = "lsum" in vset and "nosum" not in vset
                if segs_c:
                    lacc = None
                    covl = np.zeros(CHUNK, dtype=bool)
                    if use_lsum:
                        lacc = stg_pool.tile([128, CHUNK], f16, tag="lacc",
                                             bufs=2, name=f"lacc{h}_{c}")
                    if l_ps[c] is None and "nosum" not in vset:
                        l_ps[c] = ps_pool.tile(
                            [1, CHUNK], f32, tag="l",
                            bufs=1 if "psum3" in vset else 2,
                            name=f"l{h}_{c}")
                    if ot_ps[c] is None and "nopv" not in vset:
                        ot_ps[c] = ps_pool.tile(
                            [128, CHUNK], f32, tag="ot",
                            bufs=3 if "psum3" in vset else 2,
                            name=f"ot{h}_{c}")
                    cov = np.zeros(CHUNK, dtype=bool)
                    for (T, _, _, r0, r1) in segs_c:
                        a = T.q0 * BS + r0 - c * CHUNK
                        cov[a:a + (r1 - r0)] = True
                    nseg = len(segs_c)
                    assert nseg == n_ot[c], (c, nseg, n_ot[c])
                    timing_variant = bool(vset & {
                        "nosum", "nopv", "nomask", "noexp", "noqk", "noout"})
                    # PSUM has_written is per-element: the first matmul of the
                    # chunk issues start=True (clears the bank's bits); later
                    # segments overwrite where clear, accumulate where set —
                    # no explicit zero-init needed as long as every column is
                    # covered by some segment.
                    acc_first = True
                    if not cov.all() and not timing_variant:
                        # degenerate pattern: zero-fill so the evac reads no
                        # junk in never-written columns
                        if not use_lsum:
                            nc.tensor.matmul(
                                l_ps[c][:, :], zerosc[:, :1], zerosc[:],
                                start=True, stop=False, skip_group_check=True)
                        nc.tensor.matmul(
                            ot_ps[c][:, :], zerosc[:, :128], zerosc[:],
                            start=True, stop=False, skip_group_check=True)
                        acc_first = False
                    for si, (T, pgrp, pf16_, r0, r1) in enumerate(segs_c):
                        col0 = T.q0 * BS + r0 - c * CHUNK
                        vrel = T.t % (TS // 128)
                        frag = (r1 - r0) < 256 and pf16_ is not None
                        if "f16pv" in vset:
                            ones_op = ones16
                            v_op = v16_tiles[h][T.t // (TS // 128)]
                            p_op = pgrp
                        elif frag:
                            ones_op = ones16
                            v_op = v16_tiles[h][T.t // (TS // 128)]
                            p_op = pf16_
                        elif not frag:
                            ones_op = ones
                            v_op = v_tiles[h][T.t // (TS // 128)]
                            p_op = pgrp
                        last = (si == nseg - 1)
                        if use_lsum:
                            # accumulate P on DVE; one l matmul per chunk
                            a = col0
                            while a < col0 + (r1 - r0):
                                st = bool(covl[a])
                                b = a
                                while (b < col0 + (r1 - r0)
                                       and bool(covl[b]) == st):
                                    b += 1
                                src = p_op[:, T.slot, r0 + (a - col0):
                                           r0 + (b - col0)]
                                if st:
                                    nc.vector.tensor_add(
                                        lacc[:, a:b], lacc[:, a:b], src)
                                else:
                                    nc.vector.tensor_copy(lacc[:, a:b], src)
                                a = b
                            covl[col0:col0 + (r1 - r0)] = True
                        elif "nosum" not in vset:
                            nc.tensor.matmul(
                                l_ps[c][:, col0:col0 + (r1 - r0)],
                                ones_op[:],
                                p_op[:, T.slot, r0:r1],
                                start=(acc_first and si == 0), stop=last,
                                skip_group_check=True)
                        if "nopv" not in vset:
                            nc.tensor.matmul(
                                ot_ps[c][:, col0:col0 + (r1 - r0)],
                                v_op[:, vrel, :],
                                p_op[:, T.slot, r0:r1],
                                start=(acc_first and si == 0), stop=last,
                                skip_group_check=True)
                    if use_lsum:
                        a = 0
                        while a < CHUNK:           # zero never-covered runs
                            if covl[a]:
                                a += 1
                                continue
                            b = a
                            while b < CHUNK and not covl[b]:
                                b += 1
                            nc.vector.memset(lacc[:, a:b], 0.0)
                            a = b
                        nc.tensor.matmul(
                            l_ps[c][:, :], ones16[:], lacc[:, :],
                            start=True, stop=True, skip_group_check=True)

                # evacuate O^T and l for chunk c
                if (ot_ps[c] is not None
                        and not (vset & {"nopv", "noout"})):
                    ot_stage = stg_pool.tile([128, CHUNK], otdt, tag="ots",
                                             name=f"ots{h}_{c}")
                    nc.vector.tensor_copy(ot_stage[:], ot_ps[c][:])
                    nc.sync.dma_start(OT[h][:, c * CHUNK:(c + 1) * CHUNK],
                                      ot_stage[:])
                if l_ps[c] is not None and l_strip is not None:
                    nc.vector.tensor_copy(
                        l_strip[:, c * CHUNK:(c + 1) * CHUNK], l_ps[c][:])
                if "dveload" in vset:
                    scr = stg_pool.tile([128, CHUNK], f16, tag="dvescr",
                                        bufs=2, name=f"scr{h}_{c}")
                    for _ in range(4):
                        nc.vector.tensor_copy(scr[:], pconst[:, 0, :])
            if l_strip is not None:
                nc.sync.dma_start(
                    LOUT[h].rearrange("(r c) -> r c", r=1), l_strip[:])

    nc.compile()
    return nc


# ----------------------------------------------------------------------------
# host entry point
# ----------------------------------------------------------------------------

def _host_fallback(out, q, k, v, kidx, kvalid, blocks):
    """Reference-formula recompute for q-blocks with no usable pattern."""
    b, s, h, d = q.shape
    nb = s // BS
    kmax = kidx.shape[1]
    kb = k.reshape(nb, BS, h, d)
    vb = v.reshape(nb, BS, h, d)
    scale = 1.0 / math.sqrt(d)
    for i in blocks:
        qb = q[0, i * BS:(i + 1) * BS]                       # [BS, h, d]
        kg = kb[kidx[i]]                                     # [kmax, BS, h, d]
        vg = vb[kidx[i]]
        scores = np.einsum("ahd,kchd->hakc", qb, kg) * scale
        qpos = i * BS + np.arange(BS)
        kpos = kidx[i][:, None] * BS + np.arange(BS)[None, :]
        ok = (qpos[:, None, None] >= kpos[None, :, :]) & \
            kvalid[i][None, :, None]
        scores = np.where(ok[None], scores, NEG)
        sc = scores.reshape(h, BS, kmax * BS)
        sc = sc - sc.max(axis=-1, keepdims=True)
        e = np.exp(sc)
        p = e / e.sum(axis=-1, keepdims=True)
        o = np.einsum("hak,khd->ahd", p,
                      vg.reshape(kmax * BS, h, d))
        out[0, i * BS:(i + 1) * BS] = o


def _prepare(q, k, v, kidx, kvalid, n_cores):
    """Build the device program + per-core input maps."""
    b, s, h, d = q.shape
    assert b == 1 and d == 128 and s % CHUNK == 0
    hpc = h // n_cores
    nb = s // BS

    kidx = np.asarray(kidx, dtype=np.int32)
    kvalid = np.asarray(kvalid, dtype=bool)

    allow = _build_allow(kidx, kvalid, nb)
    fallback = [i for i in range(nb) if not allow[i].any()]

    tiles, groups, n_ot = _build_schedule(allow, nb, s)
    nc = _emit_program(groups, n_ot, s, hpc, n_cores)

    masklib, combos = _build_consts()
    ones = np.ones((128, 1), np.float32)

    vset = set(os.environ.get("K_VARIANT", "f16pv,lsum,lag1").split(","))
    f16mode = "f16pv" in vset
    qkdt = np.float16 if f16mode else np.float32
    if "qk8" in vset:
        qkdt = ml_dtypes.float8_e4m3
    pvdt = ml_dtypes.float8_e4m3 if "pv8" in vset else np.float16
    in_maps = []
    for c in range(n_cores):
        hs = slice(c * hpc, (c + 1) * hpc)
        qh = q[0, :, hs, :]                                  # [s, hpc, d]
        kh = k[0, :, hs, :]
        vh = v[0, :, hs, :]
        # V rearranged to [hpc, p=128, a=s//128, d] so device DMA is contiguous
        v16 = np.ascontiguousarray(
            vh.transpose(1, 0, 2).reshape(hpc, s // 128, 128, d)
            .transpose(0, 2, 1, 3)).astype(pvdt)
        m = {
            "QT": np.ascontiguousarray(
                qh.transpose(1, 2, 0)).astype(qkdt),             # [hpc, d, s]
            "KT": np.ascontiguousarray(
                kh.transpose(1, 2, 0)).astype(qkdt),
            "MASKLIB": masklib,
            "COMBOS": combos,
            "ONES": ones,
            "ONES16": np.ones((128, 1), pvdt),
            "VF16": v16,
        }
        if "dvemask" in vset:
            m["MASKC"] = _build_maskc()
        if vset & {"noqk", "noexp"}:
            m["PCONST"] = np.ones((128, 2, CHUNK), np.float16)
        if _need_zerofill(groups, s):
            m["ZEROSC"] = np.zeros((128, CHUNK), np.float32)
        if not f16mode:
            m["V"] = np.ascontiguousarray(vh.transpose(1, 0, 2))  # [hpc, s, d]
        in_maps.append(m)
    return nc, in_maps, fallback


def _postprocess(results, q, k, v, kidx, kvalid, fallback, n_cores):
    b, s, h, d = q.shape
    hpc = h // n_cores
    out = np.empty((b, s, h, d), dtype=np.float32)
    for c in range(n_cores):
        for hh in range(hpc):
            ot = results[c]["OT"][hh].astype(np.float32)     # [d, s]
            l = results[c]["LOUT"][hh]                       # [s]
            out[0, :, c * hpc + hh, :] = (ot / l[None, :]).T
    if fallback:
        _host_fallback(out, q, k, v, np.asarray(kidx, np.int32),
                       np.asarray(kvalid, bool), fallback)
    return out


def _attention_forward(q, k, v, kidx, kvalid, n_cores):
    from concourse import bass_utils

    nc, in_maps, fallback = _prepare(q, k, v, kidx, kvalid, n_cores)
    res = bass_utils.run_bass_kernel_spmd(
        nc, in_maps, core_ids=list(range(n_cores)))
    out = _postprocess(res.results, q, k, v, kidx, kvalid, fallback, n_cores)
    if res.exec_time_ns is not None:
        print(f"HW exec time: {res.exec_time_ns} ns")
    return out


def kernel(q, k, v, kidx, kvalid):
    return _attention_forward(
        np.asarray(q, dtype=np.float32), np.asarray(k, dtype=np.float32),
        np.asarray(v, dtype=np.float32), np.asarray(kidx),
        np.asarray(kvalid), n_cores=8)



# revision 48
# speedup vs baseline: 1.5218x; 1.5218x over previous
"""Trainium2 Bass kernel for DKernelPredefinedSparseAttention.

Problem: B=1, S=8192, H=16, D=128 attention where each 64-wide query block
attends to <=8 key blocks given by kidx/kvalid (block-sparse pattern with
element-level causal masking inside blocks), softmax over the gathered keys.

Strategy (host-specialized):
  - Heads sharded across 8 cores (2 heads/core), SPMD program.
  - Host pre-transposes Q and K per head to [D, S] layout (d on partitions)
    so no on-chip transposes are needed; V stays s-major.
  - kidx/kvalid are host-visible => the matmul schedule is specialized to the
    pattern: k-blocks are processed in PAIRS (2t, 2t+1) stacked on the 128
    partitions; for each pair the set of attending q-blocks forms contiguous
    runs => big moving-operand matmuls (fp32r, 1 cyc/row at N>=256).
  - Scores are computed transposed: S^T[kpos, q] (kpos on partitions), the
    causal-diagonal / invalid-block masks are ADDED via tiny bf16 matmuls
    from a host-built mask library (avoids DVE passes), exp runs on ScalarE
    (PSUM->SBUF, scale=1/sqrt(D) folded in), and PV accumulates O^T[d, q] in
    PSUM with V pairs as the stationary operand.
  - Softmax denominator (lsum, default): per chunk, the P tiles are first
    accumulated on DVE (f16 copy/add into an SBUF strip; DVE has large
    slack), then ONE ones-vector matmul per chunk reduces across kpos --
    cutting the PE columns spent on l ~4.4x vs a ones-matmul per segment
    (~40us/iter measured win, numerically identical output).
  - Unnormalized O^T and the denominators l are DMA'd out; the host does the
    final transpose + division during output assembly.
"""

import math
import os
import numpy as np
import ml_dtypes

BS = 64          # sparse block size (k-block width, q-block height)
CHUNK = 512      # q columns per PSUM accumulator bank (8 q-blocks)
TS = 2048        # SBUF tensor-tile width (s positions per k/q/v tile)
NEG = -1.0e30


# ----------------------------------------------------------------------------
# host-side schedule construction
# ----------------------------------------------------------------------------

class _Tile:
    __slots__ = ("t", "q0", "q1", "width", "start_chunk", "masks",
                 "qk_pieces", "segments", "slot", "gidx")

    def __init__(self, t, q0, q1):
        self.t = t
        self.q0 = q0            # first q-block (inclusive)
        self.q1 = q1            # last q-block (inclusive)
        self.width = (q1 - q0 + 1) * BS
        self.start_chunk = (q0 * BS) // CHUNK
        self.masks = []         # (rel_block, combo_idx)
        self.qk_pieces = []     # (abs_c0, abs_c1)
        self.segments = []      # (chunk, rel0, rel1)  [rel = col within tile]


_COMBOS = [("D", "Z"), ("Z", "D"), ("D", "I"), ("I", "D"),
           ("I", "Z"), ("Z", "I"), ("I", "I")]
_COMBO_IDX = {c: i for i, c in enumerate(_COMBOS)}


def _build_consts():
    """Mask library lhsT [128,128] and combo selector rhs [128, 64*ncombo]."""
    lib = np.zeros((128, 128), np.float32)
    for r in range(63):            # row r: top-diag mask column r
        lib[r, :64] = np.where(np.arange(64) <= r, 0.0, NEG)
    for r in range(63, 126):       # row r: bottom-diag mask column r-63
        c = r - 63
        lib[r, 64:] = np.where(np.arange(64) <= c, 0.0, NEG)
    lib[126, :64] = NEG            # top-inf
    lib[127, 64:] = NEG            # bottom-inf

    sel = np.zeros((128, 64 * len(_COMBOS)), np.float32)
    for ci, (top, bot) in enumerate(_COMBOS):
        for c in range(64):
            col = ci * 64 + c
            if top == "D":
                if c < 63:
                    sel[c, col] = 1.0
            elif top == "I":
                sel[126, col] = 1.0
            if bot == "D":
                if c < 63:
                    sel[63 + c, col] = 1.0
            elif bot == "I":
                sel[127, col] = 1.0
    return (lib.astype(ml_dtypes.bfloat16), sel.astype(ml_dtypes.bfloat16))


def _build_maskc():
    """Additive mask table for the DVE path: [128 kpos, combo, 64 qcol]."""
    m = np.zeros((128, len(_COMBOS), BS), np.float32)
    pb = np.arange(64)
    for ci, (top, bot) in enumerate(_COMBOS):
        for half, st in ((0, top), (64, bot)):
            if st == "D":
                m[half:half + 64, ci, :] = np.where(
                    pb[:, None] > np.arange(BS)[None, :], NEG, 0.0)
            elif st == "I":
                m[half:half + 64, ci, :] = NEG
    return m


def _need_zerofill(groups, s):
    """True only for degenerate patterns where some chunk's PSUM columns
    are not fully covered by segments (the evac would read junk)."""
    nchunk = s // CHUNK
    cov = [np.zeros(CHUNK, dtype=bool) for _ in range(nchunk)]
    anyseg = [False] * nchunk
    for gs in groups:
        for g in gs:
            for T in g:
                for (ch, r0, r1) in T.segments:
                    a = T.q0 * BS + r0 - ch * CHUNK
                    cov[ch][a:a + (r1 - r0)] = True
                    anyseg[ch] = True
    return any(anyseg[c] and not cov[c].all() for c in range(nchunk))


def _contiguous_runs(mask):
    runs = []
    i = 0
    n = len(mask)
    while i < n:
        if mask[i]:
            j = i
            while j + 1 < n and mask[j + 1]:
                j += 1
            runs.append((i, j))
            i = j + 1
        else:
            i += 1
    return runs


def _build_allow(kidx, kvalid, nb):
    allow = np.zeros((nb, nb), dtype=bool)
    kmax = kidx.shape[1]
    for i in range(nb):
        for jj in range(kmax):
            if kvalid[i, jj]:
                j = int(kidx[i, jj])
                if 0 <= j <= i:
                    allow[i, j] = True
    return allow


def _build_schedule(allow, nb, s):
    """Build the tile list + per-chunk grouping for one head (pattern is
    shared by all heads)."""
    chunkb = CHUNK // BS          # q-blocks per chunk
    nchunk = s // CHUNK
    tiles = []
    for t in range(nb // 2):
        j0, j1 = 2 * t, 2 * t + 1
        rows = allow[:, j0] | (allow[:, j1] if j1 < nb else False)
        for (a, b) in _contiguous_runs(list(rows)):
            if (b - a + 1) > chunkb:
                p = a
                while p <= b:
                    pe = min(b, (p // chunkb + 1) * chunkb - 1)
                    tiles.append(_Tile(t, p, pe))
                    p = pe + 1
            else:
                tiles.append(_Tile(t, a, b))

    for T in tiles:
        # masks per q-block
        for q in range(T.q0, T.q1 + 1):
            states = []
            for h in range(2):
                j = 2 * T.t + h
                if j >= nb or not allow[q, j]:
                    states.append("I")
                elif j == q:
                    states.append("D")
                else:
                    states.append("Z")
            if states != ["Z", "Z"]:
                T.masks.append((q - T.q0, _COMBO_IDX[tuple(states)]))
        # qk pieces: split [q0*BS, (q1+1)*BS) at the TS grid
        c0 = T.q0 * BS
        c1 = (T.q1 + 1) * BS
        p = c0
        while p < c1:
            pe = min(c1, (p // TS + 1) * TS)
            T.qk_pieces.append((p, pe))
            p = pe
        # segments: split at the CHUNK grid (for l and O^T accumulation)
        p = c0
        while p < c1:
            pe = min(c1, (p // CHUNK + 1) * CHUNK)
            T.segments.append((p // CHUNK, p - c0, pe - c0))
            p = pe

    # group tiles per start chunk (pairs, matched widths when possible)
    by_chunk = [[] for _ in range(nchunk)]
    for T in tiles:
        by_chunk[T.start_chunk].append(T)
    groups = [[] for _ in range(nchunk)]
    for c in range(nchunk):
        ts_sorted = sorted(by_chunk[c], key=lambda T: -T.width)
        for i in range(0, len(ts_sorted), 2):
            g = ts_sorted[i:i + 2]
            for slot, T in enumerate(g):
                T.slot = slot
            groups[c].append(g)

    # contribution counts per chunk (same for O^T and l)
    n_ot = [0] * nchunk
    for c in range(nchunk):
        for g in groups[c]:
            for T in g:
                for (ch, r0, r1) in T.segments:
                    n_ot[ch] += 1
    return tiles, groups, n_ot


# ----------------------------------------------------------------------------
# device program emission
# ----------------------------------------------------------------------------

def _emit_program(groups, n_ot, s, hpc, n_cores, repeat=1):
    import concourse.bacc as bacc
    import concourse.tile as tile
    import concourse.mybir as mybir
    from contextlib import ExitStack

    f32 = mybir.dt.float32
    f32r = mybir.dt.float32r
    bf16 = mybir.dt.bfloat16
    f16 = mybir.dt.float16
    Exp = mybir.ActivationFunctionType.Exp

    nchunk = s // CHUNK
    nt = math.ceil(s / TS)            # tensor tiles per head
    nlblk = math.ceil(nchunk / 4)
    d = 128
    scale = 1.0 / math.sqrt(float(d))

    variant = os.environ.get("K_VARIANT", "f16pv,lsum,lag1,deep")
    vset = set(variant.split(","))
    nc = bacc.Bacc("TRN2", debug=False, num_devices=n_cores)
    f8 = mybir.dt.float8e4
    qkdt = f16 if "f16pv" in vset else f32r
    if "qk8" in vset:
        qkdt = f8
    QT = nc.dram_tensor("QT", [hpc, d, s], qkdt, kind="ExternalInput").ap()
    KT = nc.dram_tensor("KT", [hpc, d, s], qkdt, kind="ExternalInput").ap()
    V = None
    if "f16pv" not in vset:
        V = nc.dram_tensor("V", [hpc, s, d], f32r, kind="ExternalInput").ap()
    # consts only some variants need: keep them out of the startup DMA
    # critical path otherwise
    need_pconst = bool(vset & {"noqk", "noexp"})
    need_maskc = "dvemask" in vset
    need_zero = _need_zerofill(groups, s)

    MASKLIB = nc.dram_tensor("MASKLIB", [128, 128], bf16, kind="ExternalInput").ap()
    MASKC = None
    if need_maskc:
        MASKC = nc.dram_tensor("MASKC", [128, len(_COMBOS), BS], f32,
                               kind="ExternalInput").ap()
    COMBOS = nc.dram_tensor("COMBOS", [128, 64 * len(_COMBOS)], bf16,
                            kind="ExternalInput").ap()
    ONES = nc.dram_tensor("ONES", [128, 1], f32r, kind="ExternalInput").ap()
    ZEROSC = None
    if need_zero:
        ZEROSC = nc.dram_tensor("ZEROSC", [128, CHUNK], f32r,
                                kind="ExternalInput").ap()
    # pre-rearranged on host to [hpc, p=128, a, d] so the DMA is contiguous
    pvdt = f8 if "pv8" in vset else f16
    VF16 = nc.dram_tensor("VF16", [hpc, 128, s // 128, d], pvdt,
                          kind="ExternalInput").ap()
    ONES16 = nc.dram_tensor("ONES16", [128, 1], pvdt,
                            kind="ExternalInput").ap()
    PCONST = None
    if need_pconst:
        PCONST = nc.dram_tensor("PCONST", [128, 2, CHUNK], f16,
                                kind="ExternalInput").ap()
    otdt = f16 if "f16pv" in vset else f32
    OT = nc.dram_tensor("OT", [hpc, d, s], otdt, kind="ExternalOutput").ap()
    LOUT = nc.dram_tensor("LOUT", [hpc, s], f32, kind="ExternalOutput").ap()

    # ---- load plan on the global (head, chunk) axis ----
    # first need (global chunk) per (head, kind, tile_n)
    first_need = {}
    for h in range(hpc):
        for c in range(nchunk):
            gc = h * nchunk + c
            for g in groups[c]:
                for T in g:
                    for (p0, p1) in T.qk_pieces:
                        key = (h, "q", p0 // TS)
                        first_need.setdefault(key, gc)
                    first_need.setdefault((h, "k", (T.t * 128) // TS), gc)
                    first_need.setdefault((h, "v", (T.t * 128) // TS), gc)
    dist = 1
    for v_ in vset:
        if v_.startswith("dist"):
            dist = int(v_[4:])
    loads_at = [[] for _ in range(hpc * nchunk)]
    for (h, kind, n), gc in sorted(first_need.items(), key=lambda kv: kv[1]):
        loads_at[max(0, gc - dist)].append((h, kind, n))
    if "prefetchall" in vset:
        loads_at = [sum(loads_at, [])] + [[] for _ in range(hpc * nchunk - 1)]

    with tile.TileContext(nc) as tc, ExitStack() as ctx:
        const_pool = ctx.enter_context(tc.tile_pool(name="consts", bufs=1))
        nres = hpc * math.ceil(s / TS)      # tiles per kind if fully resident
        pall = "prefetchall" in vset
        kq_pool = ctx.enter_context(tc.tile_pool(
            name="kq", bufs=nres if pall else 5))
        q_pool = ctx.enter_context(tc.tile_pool(
            name="qp", bufs=nres if pall else (5 if "loadonce" in vset else 3)))
        v_pool = ctx.enter_context(tc.tile_pool(
            name="vp", bufs=nres if pall else 5))
        pt_pool = ctx.enter_context(tc.tile_pool(
            name="pt", bufs=8 if "lag1" in vset
            else (6 if "deep" in vset else 4)))
        stg_pool = ctx.enter_context(tc.tile_pool(
            name="stg", bufs=4 if "deep" in vset else 2))
        ps_pool = ctx.enter_context(tc.tile_pool(name="ps", bufs=1, space="PSUM"))

        masklib = const_pool.tile([128, 128], bf16)
        combos = const_pool.tile([128, 64 * len(_COMBOS)], bf16)
        ones = const_pool.tile([128, 1], f32r)
        ones16 = const_pool.tile([128, 1], pvdt)
        nc.sync.dma_start(masklib[:], MASKLIB)
        nc.sync.dma_start(combos[:], COMBOS)
        nc.sync.dma_start(ones[:], ONES)
        nc.sync.dma_start(ones16[:], ONES16)
        zerosc = pconst = maskc = None
        if need_zero:
            zerosc = const_pool.tile([128, CHUNK], f32r)
            nc.sync.dma_start(zerosc[:], ZEROSC)
        if need_pconst:
            pconst = const_pool.tile([128, 2, CHUNK], f16)
            nc.sync.dma_start(pconst[:], PCONST)
        if need_maskc:
            maskc = const_pool.tile([128, len(_COMBOS), BS], f32)
            nc.sync.dma_start(maskc[:], MASKC)

        ptdt = f16 if "f16pv" in vset else f32r
        if "pv8" in vset:
            ptdt = f8
        kt_tiles = [[None] * nt for _ in range(hpc)]
        qt_tiles = [[None] * nt for _ in range(hpc)]
        v_tiles = [[None] * nt for _ in range(hpc)]
        v16_tiles = [[None] * nt for _ in range(hpc)]

        def do_loads(gc):
            for (h, kind, n) in loads_at[gc]:
                if "loadonce" in vset:
                    done = {"k": kt_tiles, "q": qt_tiles, "v": v16_tiles}[kind]
                    if done[h][n] is not None:
                        continue
                w = min(TS, s - n * TS)
                nsplit = 4 if ("splitload" in vset and gc == 0) else 1
                if kind == "k":
                    kt = kq_pool.tile([128, w], qkdt, tag="kt", name=f"kt{h}_{n}")
                    for sp in range(nsplit):
                        a, b = sp * w // nsplit, (sp + 1) * w // nsplit
                        nc.sync.dma_start(
                            kt[:, a:b], KT[h][:, n * TS + a:n * TS + b])
                    kt_tiles[h][n] = kt
                elif kind == "q":
                    qt = q_pool.tile([128, w], qkdt, tag="qt", name=f"qt{h}_{n}")
                    for sp in range(nsplit):
                        a, b = sp * w // nsplit, (sp + 1) * w // nsplit
                        nc.sync.dma_start(
                            qt[:, a:b], QT[h][:, n * TS + a:n * TS + b])
                    qt_tiles[h][n] = qt
                else:
                    if "f16pv" not in vset:
                        vt = v_pool.tile([128, w // 128, 128], f32r, tag="vt",
                                         name=f"vt{h}_{n}")
                        src = V[h][n * TS:n * TS + w, :].rearrange(
                            "(a p) d -> p a d", p=128)
                        nc.sync.dma_start(vt[:], src)
                        v_tiles[h][n] = vt
                    a0 = (n * TS) // 128
                    vt16 = v_pool.tile([128, w // 128, 128], pvdt, tag="vt16",
                                       name=f"vt16_{h}_{n}")
                    nc.sync.dma_start(vt16[:], VF16[h][:, a0:a0 + w // 128, :])
                    v16_tiles[h][n] = vt16

        for rep in range(repeat):
          for h in range(hpc):
            ot_ps = [None] * nchunk
            l_ps = [None] * nchunk
            pending = []          # (T, pgrp) tiles with unemitted segments

            l_strip = None
            if not (vset & {"dmaonly", "nosum", "noout"}):
                l_strip = stg_pool.tile([1, s], f32, tag="lstrip", bufs=2,
                                        name=f"lstrip{h}")
            # lag1: close chunk c's PV/l block only after chunk c+1's
            # QK/exp have been emitted, so PE never stalls on the last
            # group's exp at a chunk boundary
            lag = 1 if "lag1" in vset else 0
            for c in range(nchunk + lag):
                if c < nchunk:
                    do_loads(h * nchunk + c)
                if "dmaonly" in vset:
                    continue

                for g in (groups[c] if c < nchunk else []):
                    sgrp = None
                    if "noqk" not in vset:
                        sgrp = ps_pool.tile([128, 2, CHUNK], f32, tag="sg",
                                            bufs=2, name=f"sg{h}_{c}")
                    pgrp = None
                    if "noexp" not in vset:
                        pgrp = pt_pool.tile([128, 2, CHUNK], ptdt, tag="pg",
                                            name=f"pg{h}_{c}")

                    def _mm(T):
                        """Masks needing a PE matmul (vs DVE memset on P)."""
                        if vset & {"nomask", "noqk", "dvemask"}:
                            return []
                        if "zmask" in vset:
                            return [m for m in T.masks
                                    if "D" in _COMBOS[m[1]]]
                        return T.masks

                    # QK pieces
                    for T in g:
                        npiece = len(T.qk_pieces)
                        if "noqk" in vset:
                            npiece = 0
                        for pi, (p0, p1) in enumerate(T.qk_pieces[:npiece]):
                            n = p0 // TS
                            rel = p0 - T.q0 * BS
                            kt = kt_tiles[h][T.t // (TS // 128)]
                            krel = (T.t * 128) % TS
                            qt = qt_tiles[h][n]
                            nc.tensor.matmul(
                                sgrp[:, T.slot, rel:rel + (p1 - p0)],
                                kt[:, krel:krel + 128],
                                qt[:, p0 - n * TS:p1 - n * TS],
                                start=(pi == 0),
                                stop=(pi == npiece - 1 and not _mm(T)),
                                skip_group_check=True)
                    # masks (shared masklib stationary)
                    zmask_list = []
                    for T in g:
                        nmask = len(T.masks)
                        if vset & {"nomask", "noqk"}:
                            nmask = 0
                        if "dvemask" in vset:
                            for (rb, ci) in T.masks[:nmask]:
                                dst = sgrp[:, T.slot, rb * BS:(rb + 1) * BS]
                                nc.vector.tensor_add(dst, dst,
                                                     maskc[:, ci, :])
                            continue
                        mm = _mm(T)
                        if nmask and "zmask" in vset:
                            zmask_list += [(T, rb, _COMBOS[ci])
                                           for (rb, ci) in T.masks
                                           if "D" not in _COMBOS[ci]]
                        for mi, (rb, ci) in enumerate(mm[:nmask]):
                            nc.tensor.matmul(
                                sgrp[:, T.slot, rb * BS:(rb + 1) * BS],
                                masklib[:],
                                combos[:, ci * BS:(ci + 1) * BS],
                                start=False, stop=(mi == len(mm) - 1),
                                skip_group_check=True)
                    # exp (one call per uniform-width sub-run)
                    src = pconst if "noqk" in vset else sgrp
                    if "noexp" in vset:
                        pgrp = pconst
                    elif (len(g) == 2 and g[0].width == g[1].width
                          and "splitexp" not in vset):
                        w = g[0].width
                        nc.scalar.activation(pgrp[:, :, :w], src[:, :, :w],
                                             Exp, scale=scale)
                    else:
                        for T in g:
                            nc.scalar.activation(
                                pgrp[:, T.slot, :T.width],
                                src[:, T.slot, :T.width], Exp, scale=scale)
                    # pure-invalid halves: zero P on DVE instead of a -inf
                    # mask matmul on PE (exp of the unmasked scores is junk
                    # but finite; zeroing excludes it from l and PV)
                    if "noexp" not in vset:
                        for (T, rb, (top, bot)) in zmask_list:
                            c0, c1 = rb * BS, (rb + 1) * BS
                            if top == "I" and bot == "I":
                                nc.vector.memset(
                                    pgrp[:, T.slot, c0:c1], 0.0)
                            elif top == "I":
                                nc.vector.memset(
                                    pgrp[0:64, T.slot, c0:c1], 0.0)
                            elif bot == "I":
                                nc.vector.memset(
                                    pgrp[64:128, T.slot, c0:c1], 0.0)
                    pf16 = None
                    needs16 = [T for T in g
                               if any((r1 - r0) < 256 and len(T.segments) > 1
                                      for (ch, r0, r1) in T.segments)]
                    if "f16pv" in vset:
                        needs16 = []
                    if needs16:
                        pf16 = pt_pool.tile([128, 2, CHUNK], f16, tag="pf16",
                                            bufs=4, name=f"pf16_{h}_{c}")
                        for T in needs16:
                            nc.vector.tensor_copy(
                                pf16[:, T.slot, :T.width],
                                pgrp[:, T.slot, :T.width])
                    for T in g:
                        pending.append((T, pgrp, pf16))

                # ---- close chunk c-lag: emit its l/PV segments ----
                c = c - lag
                if c < 0:
                    continue
                segs_c = []
                for (T, pgrp, pf16_) in pending:
                    for (ch, r0, r1) in T.segments:
                        if ch == c:
                            segs_c.append((T, pgrp, pf16_, r0, r1))
                pending = [(T, p, pf) for (T, p, pf) in pending
                           if any(ch > c for (ch, _, _) in T.segments)]
                if "segsort" in vset:
                    # widest first: the lsum DVE pass then does one
                    # full-width copy + one add per later segment
                    segs_c.sort(key=lambda s: -(s[4] - s[3]))

                use_lsum = "lsum" in vset and "nosum" not in vset
                if segs_c:
                    lacc = None
                    covl = np.zeros(CHUNK, dtype=bool)
                    if use_lsum:
                        lacc = stg_pool.tile([128, CHUNK], f16, tag="lacc",
                                             bufs=2, name=f"lacc{h}_{c}")
                    if l_ps[c] is None and "nosum" not in vset:
                        l_ps[c] = ps_pool.tile(
                            [1, CHUNK], f32, tag="l",
                            bufs=1 if "psum3" in vset else 2,
                            name=f"l{h}_{c}")
                    if ot_ps[c] is None and "nopv" not in vset:
                        ot_ps[c] = ps_pool.tile(
                            [128, CHUNK], f32, tag="ot",
                            bufs=3 if "psum3" in vset else 2,
                            name=f"ot{h}_{c}")
                    cov = np.zeros(CHUNK, dtype=bool)
                    for (T, _, _, r0, r1) in segs_c:
                        a = T.q0 * BS + r0 - c * CHUNK
                        cov[a:a + (r1 - r0)] = True
                    nseg = len(segs_c)
                    assert nseg == n_ot[c], (c, nseg, n_ot[c])
                    timing_variant = bool(vset & {
                        "nosum", "nopv", "nomask", "noexp", "noqk", "noout"})
                    # PSUM has_written is per-element: the first matmul of the
                    # chunk issues start=True (clears the bank's bits); later
                    # segments overwrite where clear, accumulate where set —
                    # no explicit zero-init needed as long as every column is
                    # covered by some segment.
                    acc_first = True
                    if not cov.all() and not timing_variant:
                        # degenerate pattern: zero-fill so the evac reads no
                        # junk in never-written columns
                        if not use_lsum:
                            nc.tensor.matmul(
                                l_ps[c][:, :], zerosc[:, :1], zerosc[:],
                                start=True, stop=False, skip_group_check=True)
                        nc.tensor.matmul(
                            ot_ps[c][:, :], zerosc[:, :128], zerosc[:],
                            start=True, stop=False, skip_group_check=True)
                        acc_first = False
                    for si, (T, pgrp, pf16_, r0, r1) in enumerate(segs_c):
                        col0 = T.q0 * BS + r0 - c * CHUNK
                        vrel = T.t % (TS // 128)
                        frag = (r1 - r0) < 256 and pf16_ is not None
                        if "f16pv" in vset:
                            ones_op = ones16
                            v_op = v16_tiles[h][T.t // (TS // 128)]
                            p_op = pgrp
                        elif frag:
                            ones_op = ones16
                            v_op = v16_tiles[h][T.t // (TS // 128)]
                            p_op = pf16_
                        elif not frag:
                            ones_op = ones
                            v_op = v_tiles[h][T.t // (TS // 128)]
                            p_op = pgrp
                        last = (si == nseg - 1)
                        if use_lsum:
                            # accumulate P on DVE; one l matmul per chunk
                            a = col0
                            while a < col0 + (r1 - r0):
                                st = bool(covl[a])
                                b = a
                                while (b < col0 + (r1 - r0)
                                       and bool(covl[b]) == st):
                                    b += 1
                                src = p_op[:, T.slot, r0 + (a - col0):
                                           r0 + (b - col0)]
                                if st:
                                    nc.vector.tensor_add(
                                        lacc[:, a:b], lacc[:, a:b], src)
                                else:
                                    nc.vector.tensor_copy(lacc[:, a:b], src)
                                a = b
                            covl[col0:col0 + (r1 - r0)] = True
                        elif "nosum" not in vset:
                            nc.tensor.matmul(
                                l_ps[c][:, col0:col0 + (r1 - r0)],
                                ones_op[:],
                                p_op[:, T.slot, r0:r1],
                                start=(acc_first and si == 0), stop=last,
                                skip_group_check=True)
                        if "nopv" not in vset:
                            nc.tensor.matmul(
                                ot_ps[c][:, col0:col0 + (r1 - r0)],
                                v_op[:, vrel, :],
                                p_op[:, T.slot, r0:r1],
                                start=(acc_first and si == 0), stop=last,
                                skip_group_check=True)
                    if use_lsum:
                        a = 0
                        while a < CHUNK:           # zero never-covered runs
                            if covl[a]:
                                a += 1
                                continue
                            b = a
                            while b < CHUNK and not covl[b]:
                                b += 1
                            nc.vector.memset(lacc[:, a:b], 0.0)
                            a = b
                        nc.tensor.matmul(
                            l_ps[c][:, :], ones16[:], lacc[:, :],
                            start=True, stop=True, skip_group_check=True)

                # evacuate O^T and l for chunk c
                if (ot_ps[c] is not None
                        and not (vset & {"nopv", "noout"})):
                    ot_stage = stg_pool.tile([128, CHUNK], otdt, tag="ots",
                                             name=f"ots{h}_{c}")
                    nc.vector.tensor_copy(ot_stage[:], ot_ps[c][:])
                    nc.sync.dma_start(OT[h][:, c * CHUNK:(c + 1) * CHUNK],
                                      ot_stage[:])
                if l_ps[c] is not None and l_strip is not None:
                    nc.vector.tensor_copy(
                        l_strip[:, c * CHUNK:(c + 1) * CHUNK], l_ps[c][:])
                if "dveload" in vset:
                    scr = stg_pool.tile([128, CHUNK], f16, tag="dvescr",
                                        bufs=2, name=f"scr{h}_{c}")
                    for _ in range(4):
                        nc.vector.tensor_copy(scr[:], pconst[:, 0, :])
            if l_strip is not None:
                nc.sync.dma_start(
                    LOUT[h].rearrange("(r c) -> r c", r=1), l_strip[:])

    nc.compile()
    return nc


# ----------------------------------------------------------------------------
# host entry point
# ----------------------------------------------------------------------------

def _host_fallback(out, q, k, v, kidx, kvalid, blocks):
    """Reference-formula recompute for q-blocks with no usable pattern."""
    b, s, h, d = q.shape
    nb = s // BS
    kmax = kidx.shape[1]
    kb = k.reshape(nb, BS, h, d)
    vb = v.reshape(nb, BS, h, d)
    scale = 1.0 / math.sqrt(d)
    for i in blocks:
        qb = q[0, i * BS:(i + 1) * BS]                       # [BS, h, d]
        kg = kb[kidx[i]]                                     # [kmax, BS, h, d]
        vg = vb[kidx[i]]
        scores = np.einsum("ahd,kchd->hakc", qb, kg) * scale
        qpos = i * BS + np.arange(BS)
        kpos = kidx[i][:, None] * BS + np.arange(BS)[None, :]
        ok = (qpos[:, None, None] >= kpos[None, :, :]) & \
            kvalid[i][None, :, None]
        scores = np.where(ok[None], scores, NEG)
        sc = scores.reshape(h, BS, kmax * BS)
        sc = sc - sc.max(axis=-1, keepdims=True)
        e = np.exp(sc)
        p = e / e.sum(axis=-1, keepdims=True)
        o = np.einsum("hak,khd->ahd", p,
                      vg.reshape(kmax * BS, h, d))
        out[0, i * BS:(i + 1) * BS] = o


def _prepare(q, k, v, kidx, kvalid, n_cores):
    """Build the device program + per-core input maps."""
    b, s, h, d = q.shape
    assert b == 1 and d == 128 and s % CHUNK == 0
    hpc = h // n_cores
    nb = s // BS

    kidx = np.asarray(kidx, dtype=np.int32)
    kvalid = np.asarray(kvalid, dtype=bool)

    allow = _build_allow(kidx, kvalid, nb)
    fallback = [i for i in range(nb) if not allow[i].any()]

    tiles, groups, n_ot = _build_schedule(allow, nb, s)
    nc = _emit_program(groups, n_ot, s, hpc, n_cores)

    masklib, combos = _build_consts()
    ones = np.ones((128, 1), np.float32)

    vset = set(os.environ.get("K_VARIANT", "f16pv,lsum,lag1,deep").split(","))
    f16mode = "f16pv" in vset
    qkdt = np.float16 if f16mode else np.float32
    if "qk8" in vset:
        qkdt = ml_dtypes.float8_e4m3
    pvdt = ml_dtypes.float8_e4m3 if "pv8" in vset else np.float16
    in_maps = []
    for c in range(n_cores):
        hs = slice(c * hpc, (c + 1) * hpc)
        qh = q[0, :, hs, :]                                  # [s, hpc, d]
        kh = k[0, :, hs, :]
        vh = v[0, :, hs, :]
        # V rearranged to [hpc, p=128, a=s//128, d] so device DMA is contiguous
        v16 = np.ascontiguousarray(
            vh.transpose(1, 0, 2).reshape(hpc, s // 128, 128, d)
            .transpose(0, 2, 1, 3)).astype(pvdt)
        m = {
            "QT": np.ascontiguousarray(
                qh.transpose(1, 2, 0)).astype(qkdt),             # [hpc, d, s]
            "KT": np.ascontiguousarray(
                kh.transpose(1, 2, 0)).astype(qkdt),
            "MASKLIB": masklib,
            "COMBOS": combos,
            "ONES": ones,
            "ONES16": np.ones((128, 1), pvdt),
            "VF16": v16,
        }
        if "dvemask" in vset:
            m["MASKC"] = _build_maskc()
        if vset & {"noqk", "noexp"}:
            m["PCONST"] = np.ones((128, 2, CHUNK), np.float16)
        if _need_zerofill(groups, s):
            m["ZEROSC"] = np.zeros((128, CHUNK), np.float32)
        if not f16mode:
            m["V"] = np.ascontiguousarray(vh.transpose(1, 0, 2))  # [hpc, s, d]
        in_maps.append(m)
    return nc, in_maps, fallback


def _postprocess(results, q, k, v, kidx, kvalid, fallback, n_cores):
    b, s, h, d = q.shape
    hpc = h // n_cores
    out = np.empty((b, s, h, d), dtype=np.float32)
    for c in range(n_cores):
        for hh in range(hpc):
            ot = results[c]["OT"][hh].astype(np.float32)     # [d, s]
            l = results[c]["LOUT"][hh]                       # [s]
            out[0, :, c * hpc + hh, :] = (ot / l[None, :]).T
    if fallback:
        _host_fallback(out, q, k, v, np.asarray(kidx, np.int32),
                       np.asarray(kvalid, bool), fallback)
    return out


def _attention_forward(q, k, v, kidx, kvalid, n_cores):
    from concourse import bass_utils

    nc, in_maps, fallback = _prepare(q, k, v, kidx, kvalid, n_cores)
    res = bass_utils.run_bass_kernel_spmd(
        nc, in_maps, core_ids=list(range(n_cores)))
    out = _postprocess(res.results, q, k, v, kidx, kvalid, fallback, n_cores)
    if res.exec_time_ns is not None:
        print(f"HW exec time: {res.exec_time_ns} ns")
    return out


def kernel(q, k, v, kidx, kvalid):
    return _attention_forward(
        np.asarray(q, dtype=np.float32), np.asarray(k, dtype=np.float32),
        np.asarray(v, dtype=np.float32), np.asarray(kidx),
        np.asarray(kvalid), n_cores=8)

